# revision 1
# baseline (speedup 1.0000x reference)
"""Trainium2 Bass kernel for DepthwiseXCorrAug.

Computes, for B=64 samples sharded 8-per-core across 8 NeuronCores:
  k = relu(bn(conv3x3_valid(kernel_in, w_k)))     # [B,256,5,5]
  s = relu(bn(conv3x3_same(search_in, w_s)))      # [B,256,31,31]
  out = per-sample per-channel xcorr(s, k), pad 2 # [B,256,31,31]

Device strategy (per core):
  - everything in bf16 on the PE (weights, activations); accumulate f32 PSUM.
    (bf16 full-width matmuls run at exactly N/2.4GHz with no weight-swap
    bubble; fp32r pays +77ns/MM. fp8 gains nothing and fails numerics.)
  - conv branches as full-width (ci-block x 3x3-tap) matmuls accumulated in
    PSUM; BN folded into weights on host, bias+ReLU by ScalarE on eviction.
  - depthwise xcorr as bf16 64x64-diagonal-weight tile matmuls
    (tile_position): per (sample-pair, ob) chunk, 4 tiles x 25 taps
    accumulate in 4 PSUM banks. This path is moving-stream-bus bound
    (~240 elem/cycle in tiled mode).
  - conv_k first (smallest DMA deps -> PE starts early), conv_s pairs and
    xcorr chunks interleaved; outputs stream out as bf16 full-row DMAs
    spread across Sync/GpSimd queues; host converts to f32.
"""

import sys

sys.path.insert(0, "/opt/trn_rl_repo")

import numpy as np

import concourse.bass as bass
import concourse.mybir as mybir
import concourse.tile as tile
from concourse import bacc
from concourse.bass_utils import run_bass_kernel_spmd

EPS = 1e-5
N_CORES = 8
B, CIN, HID = 64, 256, 256
SPC = B // N_CORES  # samples per core

_cached_nc = None
last_results = None  # set by kernel(); used by test harness for profiling


def _build_program():
    f32 = mybir.dt.float32
    bf16 = mybir.dt.bfloat16
    RELU = mybir.ActivationFunctionType.Relu

    nc = bacc.Bacc("TRN2", target_bir_lowering=False, debug=False,
                   num_devices=N_CORES)

    wTs_d = [nc.dram_tensor(f"wTs{cb}", [128, 2304], bf16, kind="ExternalInput").ap()
             for cb in range(2)]
    wTk_d = [nc.dram_tensor(f"wTk{cb}", [128, 2304], bf16, kind="ExternalInput").ap()
             for cb in range(2)]
    xk_d = [nc.dram_tensor(f"xk{cb}", [128, 1800], bf16, kind="ExternalInput").ap()
            for cb in range(2)]
    xs_d = nc.dram_tensor("xs", [SPC, 128, 2 * 33 * 34], bf16, kind="ExternalInput").ap()
    bk_d = nc.dram_tensor("bk", [2, 128, 1], f32, kind="ExternalInput").ap()
    bs_d = nc.dram_tensor("bs", [2, 128, 1], f32, kind="ExternalInput").ap()
    m64rep_d = nc.dram_tensor("m64rep", [128, 1600], bf16, kind="ExternalInput").ap()
    out_d = nc.dram_tensor("out", [SPC, CIN, 31, 31], bf16, kind="ExternalOutput").ap()
    out_flat = out_d.rearrange("s c h w -> s c (h w)")

    with tile.TileContext(nc) as tc:
        with tc.tile_pool(name="wp", bufs=1) as wp, \
             tc.tile_pool(name="spin", bufs=8) as spin_pool, \
             tc.tile_pool(name="spoutp", bufs=1) as spout_pool, \
             tc.tile_pool(name="stripp", bufs=1) as strip_pool, \
             tc.tile_pool(name="xop", bufs=8) as xout_pool, \
             tc.tile_pool(name="ps", bufs=8, space="PSUM") as ps:

            # ---- persistent inputs (weights split per (cb, ob) for precise
            # DMA dependencies) ----
            wTs = {(cb, ob): wp.tile([128, 1152], bf16, tag=f"wTs{cb}{ob}",
                                     name=f"wTs{cb}{ob}")
                   for cb in range(2) for ob in range(2)}
            wTk = {(cb, ob): wp.tile([128, 1152], bf16, tag=f"wTk{cb}{ob}",
                                     name=f"wTk{cb}{ob}")
                   for cb in range(2) for ob in range(2)}
            xk = [wp.tile([128, 1800], bf16, tag=f"xk{cb}", name=f"xk{cb}")
                  for cb in range(2)]
            bk = [wp.tile([128, 1], f32, tag=f"bk{ob}", name=f"bk{ob}")
                  for ob in range(2)]
            bs = [wp.tile([128, 1], f32, tag=f"bs{ob}", name=f"bs{ob}")
                  for ob in range(2)]
            m64rep = wp.tile([128, 1600], bf16, tag="m64rep", name="m64rep")
            kf = [wp.tile([128, 200], f32, tag=f"kf{ob}", name=f"kf{ob}")
                  for ob in range(2)]

            # ---- spout tiles (bf16); zero only the 2-wide borders ----
            # 8 physical tiles, reused by samples s and s+4 (deps tracked)
            spout = {}
            for s in range(4):
                for ob in range(2):
                    sp = spout_pool.tile([128, 35 * 35], bf16,
                                         tag=f"spout{s}_{ob}", name=f"spout{s}_{ob}")
                    spout[(s, ob)] = sp
                    spout[(s + 4, ob)] = sp
                    eng = nc.vector if (s + ob) % 2 == 0 else nc.gpsimd
                    # zero the 2-wide border frame; interior is written by
                    # the conv_s activation
                    v = sp[:].rearrange("p (r c) -> p r c", r=35, c=35)
                    eng.memset(sp[:, 0:70], 0.0)
                    eng.memset(sp[:, 1155:1225], 0.0)
                    eng.memset(v[:, 2:33, 0:2], 0.0)
                    eng.memset(v[:, 2:33, 33:35], 0.0)

            # spin prefetch state
            spin_views = {}

            def prefetch_pair(pair, deng):
                s0 = pair * 2
                for s in (s0, s0 + 1):
                    t_in = spin_pool.tile([128, 2 * 33 * 34], bf16,
                                          tag="spin", name=f"spin{s}")
                    deng.dma_start(t_in[:], xs_d[s])
                    for cb in range(2):
                        spin_views[(s, cb)] = t_in[
                            :, cb * 1122:(cb + 1) * 1122].rearrange(
                            "p (h w) -> p h w", h=33, w=34)

            # ---- DMA order: conv_k deps first so PE starts ASAP; issue
            # spread across engines (sync: conv_k path, vector: conv_s path,
            # scalar: small tensors) ----
            for cb in range(2):
                nc.sync.dma_start(wTk[(cb, 0)][:], wTk_d[cb][:, 0:1152])
                for c0 in (0, 900):
                    nc.sync.dma_start(xk[cb][:, c0:c0 + 900],
                                      xk_d[cb][:, c0:c0 + 900])
            for cb in range(2):
                nc.sync.dma_start(wTk[(cb, 1)][:], wTk_d[cb][:, 1152:2304])
            prefetch_pair(0, nc.sync)
            for cb in range(2):
                nc.sync.dma_start(wTs[(cb, 0)][:], wTs_d[cb][:, 0:1152])
            for ob in range(2):
                nc.scalar.dma_start(bk[ob][:], bk_d[ob])
                nc.scalar.dma_start(bs[ob][:], bs_d[ob])
            nc.scalar.dma_start(m64rep[:], m64rep_d)
            for cb in range(2):
                nc.scalar.dma_start(wTs[(cb, 1)][:], wTs_d[cb][:, 1152:2304])
            for p in (1, 2, 3):
                prefetch_pair(p, nc.scalar)

            # ---- conv_k: all 8 samples batched on the free dim (N=256) ----
            def emit_conv_k():
                for ob in range(2):
                    pk = ps.tile([128, 512], f32, tag="mm", name=f"pk{ob}")
                    idx = 0
                    for cb in range(2):
                        for t in range(9):
                            nc.tensor.matmul(
                                pk[:, 0:200],
                                wTk[(cb, ob)][:, t * 128:(t + 1) * 128],
                                xk[cb][:, t * 200:(t + 1) * 200],
                                start=(idx == 0), stop=(idx == 17))
                            idx += 1
                    nc.scalar.activation(kf[ob][:], pk[:, 0:200], RELU,
                                         bias=bk[ob][:, 0:1], scale=1.0)

            # ---- strips: bf16 64-diag weights, one DVE op per (s, ob) ----
            # 8 physical tiles, reused by samples s and s+4
            strips = {}
            for s in range(4):
                for ob in range(2):
                    st = strip_pool.tile(
                        [128, 1600], bf16,
                        tag=f"strip{s}_{ob}", name=f"strip{s}_{ob}")
                    strips[(s, ob)] = st
                    strips[(s + 4, ob)] = st

            def emit_strips(samples, skip=frozenset()):
                for ob in range(2):
                    for s in samples:
                        if (s, ob) in skip:
                            continue
                        kfb = kf[ob][:, s * 25:(s + 1) * 25].unsqueeze(
                            -1).broadcast_to([128, 25, 64])
                        nc.vector.tensor_tensor(
                            strips[(s, ob)][:], m64rep[:], kfb,
                            mybir.AluOpType.mult)

            # ---- conv_s: one pair of samples, both ob blocks ----
            def conv_s_pair(pair):
                s0 = pair * 2
                views = spin_views
                for ob in range(2):
                    ptiles = {}
                    for s in (s0, s0 + 1):
                        for ci in range(2):
                            ptiles[(s, ci)] = ps.tile(
                                [128, 512], f32, tag="mm",
                                name=f"pc{s}_{ob}_{ci}")
                    idx = 0
                    for cb in range(2):
                        for t in range(9):
                            dy, dx = t // 3, t % 3
                            lhsT = wTs[(cb, ob)][:, t * 128:(t + 1) * 128]
                            for s in (s0, s0 + 1):
                                for ci, (y0, nr) in enumerate([(0, 16), (16, 15)]):
                                    nc.tensor.matmul(
                                        ptiles[(s, ci)][:, 0:nr * 31],
                                        lhsT,
                                        views[(s, cb)][:, y0 + dy:y0 + dy + nr,
                                                       dx:dx + 31],
                                        start=(idx == 0), stop=(idx == 17))
                            idx += 1
                    for s in (s0, s0 + 1):
                        sov = spout[(s, ob)][:].rearrange(
                            "p (h w) -> p h w", h=35, w=35)
                        for ci, (y0, nr) in enumerate([(0, 16), (16, 15)]):
                            pv = ptiles[(s, ci)][:, 0:nr * 31].rearrange(
                                "p (h w) -> p h w", h=nr, w=31)
                            nc.scalar.activation(
                                sov[:, 2 + y0:2 + y0 + nr, 2:33],
                                pv[:, :, :], RELU,
                                bias=bs[ob][:, 0:1], scale=1.0)

            # ---- xcorr: 64x64-tile chunk per (q, ob): sample pair (2q, 2q+1)
            CI_SPEC = [(0, 16), (16, 15)]

            def xcorr_chunk(q, ob):
                sovs = [spout[(q * 2 + j, ob)][:].rearrange(
                    "p (h w) -> p h w", h=35, w=35) for j in range(2)]
                px = {}
                for ci in range(2):
                    for i in range(2):
                        px[(ci, i)] = ps.tile([128, 512], f32, tag="mm",
                                              name=f"px{q}_{ob}_{ci}_{i}")
                for t in range(25):
                    dy, dx = t // 5, t % 5
                    for i in range(2):
                        for j in range(2):
                            st = strips[(q * 2 + j, ob)]
                            lhsT = st[64 * i:64 * i + 64, t * 64:(t + 1) * 64]
                            for ci, (y0, nr) in enumerate(CI_SPEC):
                                nc.tensor.matmul(
                                    px[(ci, i)][64 * j:64 * j + 64, 0:nr * 31],
                                    lhsT,
                                    sovs[j][64 * i:64 * i + 64,
                                            y0 + dy:y0 + dy + nr, dx:dx + 31],
                                    start=(t == 0), stop=(t == 24),
                                    tile_position=(64 * i, 64 * j))
                n_ev = 0
                for i in range(2):
                    xo = xout_pool.tile([128, 1024], bf16, tag="xo",
                                        name=f"xo{q}_{ob}_{i}")
                    for ci, (y0, nr) in enumerate(CI_SPEC):
                        N = nr * 31
                        if n_ev % 2 == 0:
                            nc.vector.tensor_copy(
                                xo[:, y0 * 31:y0 * 31 + N], px[(ci, i)][:, 0:N])
                        else:
                            nc.scalar.copy(
                                xo[:, y0 * 31:y0 * 31 + N], px[(ci, i)][:, 0:N])
                        n_ev += 1
                    dst = out_flat[q * 2:q * 2 + 2,
                                   ob * 128 + 64 * i:ob * 128 + 64 * i + 64,
                                   0:961]
                    deng = nc.gpsimd if (q + ob + i) % 2 == 0 else nc.sync
                    deng.dma_start(dst, xo[:, 0:961])

            emit_conv_k()
            emit_strips(range(4))
            conv_s_pair(0)
            conv_s_pair(1)
            xcorr_chunk(0, 0)
            xcorr_chunk(0, 1)
            xcorr_chunk(1, 0)
            xcorr_chunk(1, 1)
            emit_strips(range(4, SPC))
            conv_s_pair(2)
            conv_s_pair(3)
            for q in (2, 3):
                xcorr_chunk(q, 0)
                xcorr_chunk(q, 1)

    nc.compile()
    return nc


def _host_prep(kernel, search, w_k, g_k, b_k, m_k, v_k, w_s, g_s, b_s, m_s, v_s):
    import ml_dtypes
    bf16 = ml_dtypes.bfloat16

    def fold(w, g, b, m, v):
        scale = g / np.sqrt(v + EPS)
        return (w * scale[:, None, None, None]).astype(np.float32), \
               (b - m * scale).astype(np.float32)

    wkf, bias_k = fold(w_k, g_k, b_k, m_k, v_k)
    wsf, bias_s = fold(w_s, g_s, b_s, m_s, v_s)

    def packT(w):  # [o, ci, 3, 3] -> [cb, ci, (ob, t, o)] bf16
        arr = w.reshape(2, 128, 2, 128, 9).transpose(2, 3, 0, 4, 1)
        return np.ascontiguousarray(arr, dtype=np.float32).astype(
            bf16).reshape(2, 128, 2304)

    wTk = packT(wkf)
    wTs = packT(wsf)

    M64 = np.zeros((128, 64), dtype=np.float32)
    for p in range(128):
        M64[p, p % 64] = 1.0
    M64REP = np.tile(M64, (1, 25)).astype(bf16)

    bk = np.ascontiguousarray(bias_k.reshape(2, 128, 1))
    bs = np.ascontiguousarray(bias_s.reshape(2, 128, 1))

    in_maps = []
    for core in range(N_CORES):
        kin = kernel[core * SPC:(core + 1) * SPC]
        sin = search[core * SPC:(core + 1) * SPC]

        Xk = np.zeros((2, 128, 9, 200), dtype=np.float32)
        for t in range(9):
            dy, dx = t // 3, t % 3
            p = kin[:, :, dy:dy + 5, dx:dx + 5].reshape(SPC, 2, 128, 25)
            Xk[:, :, t, :] = p.transpose(1, 2, 0, 3).reshape(2, 128, 200)
        Xk = Xk.astype(bf16).reshape(2, 128, 1800)

        Xs = np.zeros((SPC, 2, 128, 33, 34), dtype=np.float32)
        Xs[:, :, :, 1:32, 1:32] = sin.reshape(SPC, 2, 128, 31, 31)
        Xs = np.ascontiguousarray(
            Xs.transpose(0, 2, 1, 3, 4)).astype(bf16).reshape(
            SPC, 128, 2 * 33 * 34)

        in_maps.append({
            "wTs0": wTs[0], "wTs1": wTs[1],
            "wTk0": wTk[0], "wTk1": wTk[1],
            "xk0": Xk[0], "xk1": Xk[1],
            "xs": Xs, "bk": bk, "bs": bs, "m64rep": M64REP,
        })
    return in_maps


def kernel(kernel, search, w_k, g_k, b_k, m_k, v_k, w_s, g_s, b_s, m_s, v_s,
           _trace=False):
    global _cached_nc, last_results
    args = [np.ascontiguousarray(np.asarray(x, dtype=np.float32)) for x in
            (kernel, search, w_k, g_k, b_k, m_k, v_k, w_s, g_s, b_s, m_s, v_s)]
    if _cached_nc is None:
        _cached_nc = _build_program()
    nc = _cached_nc
    in_maps = _host_prep(*args)
    res = run_bass_kernel_spmd(nc, in_maps, core_ids=list(range(N_CORES)),
                               trace=_trace)
    last_results = res
    out = np.concatenate([res.results[i]["out"] for i in range(N_CORES)], axis=0)
    return np.ascontiguousarray(out.astype(np.float32))



# revision 15
# speedup vs baseline: 1.1047x; 1.1047x over previous
"""Trainium2 Bass kernel for DepthwiseXCorrAug.

Computes, for B=64 samples sharded 8-per-core across 8 NeuronCores:
  k = relu(bn(conv3x3_valid(kernel_in, w_k)))     # [B,256,5,5]
  s = relu(bn(conv3x3_same(search_in, w_s)))      # [B,256,31,31]
  out = per-sample per-channel xcorr(s, k), pad 2 # [B,256,31,31]

v2 design (per core), driven by NTFF trace analysis of v1 (245-292us):
  - PE instruction stream is made strictly in-order via an explicit
    same-engine dependency chain; this makes shared-weight matmul groups
    (InstMatmult.ldweights=False) safe against scheduler interleaving.
  - ~10 junk warmup matmuls at t=0 push the PE HAM clock gate to 8/8
    (2.4GHz) before real work arrives (saves the 1.2GHz cold phase).
  - conv branches: bf16 full-width matmuls, bias+ReLU by ScalarE on
    eviction. conv_s batches 4 samples x 2 ci per weight load; only the
    first matmul of each group self-loads weights (v1 paid a ~40ns/MM
    LDWEIGHTS tax on every matmul).
  - xcorr: 16 concurrent 32x32 diagonal tiles (4 samples x 4 channel
    quarters), ONE full-width 128-col LDWEIGHTS per tap loads all 16
    diag blocks (strips for 4 samples interleaved on the free dim);
    all 16 matmuls run with ldweights=False. 2x the per-tap throughput
    of v1's 2x2 64x64 scheme and ~1/32 the weight-load instructions.
  - kf (conv_k output) held bf16 so the DVE strip build has no casts.
  - outputs staged per (chunk, ob) into [128, 3844] tiles -> 8 fat DMAs
    with ~4KB per-partition descriptors (v1: 16 DMAs, 1.9KB rows, 24us
    unoverlapped tail); host does the final layout unshuffle.
"""

import sys

sys.path.insert(0, "/opt/trn_rl_repo")

import numpy as np

import concourse.bass as bass
import concourse.mybir as mybir
import concourse.tile as tile
from concourse import bacc
from concourse.bass_utils import run_bass_kernel_spmd
from concourse.tile_rust import add_dep_helper

EPS = 1e-5
N_CORES = 8
B, CIN, HID = 64, 256, 256
SPC = B // N_CORES  # samples per core
CI_SPEC = [(0, 16), (16, 15)]  # (y0, nr) output-row split, 961 = 496+465

_cached_nc = None
last_results = None  # set by kernel(); used by test harness for profiling


def _build_program():
    f32 = mybir.dt.float32
    bf16 = mybir.dt.bfloat16
    RELU = mybir.ActivationFunctionType.Relu

    nc = bacc.Bacc("TRN2", target_bir_lowering=False, debug=False,
                   num_devices=N_CORES)

    wTs_d = [nc.dram_tensor(f"wTs{cb}", [128, 2304], bf16, kind="ExternalInput").ap()
             for cb in range(2)]
    wTk_d = [nc.dram_tensor(f"wTk{cb}", [128, 2304], bf16, kind="ExternalInput").ap()
             for cb in range(2)]
    xk_d = [nc.dram_tensor(f"xk{cb}", [128, 1800], bf16, kind="ExternalInput").ap()
            for cb in range(2)]
    xs_d = nc.dram_tensor("xs", [SPC, 128, 2 * 33 * 34], bf16, kind="ExternalInput").ap()
    bk_d = nc.dram_tensor("bk", [2, 128, 1], f32, kind="ExternalInput").ap()
    bs_d = nc.dram_tensor("bs", [2, 128, 1], f32, kind="ExternalInput").ap()
    m32_d = nc.dram_tensor("m32rep", [128, 3200], bf16, kind="ExternalInput").ap()
    # out[X, ob, p=32j+c, (ci0: 4i x 496 | ci1: 4i x 465)]
    out_d = nc.dram_tensor("out", [2, 2, 128, 3844], bf16, kind="ExternalOutput").ap()

    pe_chain = []          # strict in-order PE instruction chain (mybir insts)
    noload_pairs = []      # (loader_inst, [mm_insts]) for post-compile check

    def PE(binst):
        inst = binst.ins
        if pe_chain:
            add_dep_helper(inst, pe_chain[-1], reason="pe-inorder-chain")
        pe_chain.append(inst)
        return inst

    with tile.TileContext(nc) as tc:
        with tc.tile_pool(name="wp", bufs=1) as wp, \
             tc.tile_pool(name="spin", bufs=8) as spin_pool, \
             tc.tile_pool(name="spoutp", bufs=1) as spout_pool, \
             tc.tile_pool(name="xop", bufs=2) as xout_pool, \
             tc.tile_pool(name="psA", bufs=4, space="PSUM") as psA, \
             tc.tile_pool(name="psB", bufs=4, space="PSUM") as psB:

            # ---- persistent tiles ----
            wTs = {(cb, ob): wp.tile([128, 1152], bf16, tag=f"wTs{cb}{ob}",
                                     name=f"wTs{cb}{ob}")
                   for cb in range(2) for ob in range(2)}
            wTk = {(cb, ob): wp.tile([128, 1152], bf16, tag=f"wTk{cb}{ob}",
                                     name=f"wTk{cb}{ob}")
                   for cb in range(2) for ob in range(2)}
            xk = [wp.tile([128, 1800], bf16, tag=f"xk{cb}", name=f"xk{cb}")
                  for cb in range(2)]
            bk = [wp.tile([128, 1], f32, tag=f"bk{ob}", name=f"bk{ob}")
                  for ob in range(2)]
            bs = [wp.tile([128, 1], f32, tag=f"bs{ob}", name=f"bs{ob}")
                  for ob in range(2)]
            m32 = wp.tile([128, 3200], bf16, tag="m32", name="m32")
            kf = [wp.tile([128, 200], bf16, tag=f"kf{ob}", name=f"kf{ob}")
                  for ob in range(2)]
            wx = {(X, ob): wp.tile([128, 3200], bf16, tag=f"wx{X}{ob}",
                                   name=f"wx{X}{ob}")
                  for X in range(2) for ob in range(2)}
            warm = wp.tile([128, 640], bf16, tag="warm", name="warm")

            # ---- spout tiles: all 16 live (xcorr runs after all conv_s) ----
            spout = {}
            for s in range(SPC):
                for ob in range(2):
                    sp = spout_pool.tile([128, 35 * 35], bf16,
                                         tag=f"spout{s}_{ob}", name=f"spout{s}_{ob}")
                    spout[(s, ob)] = sp
                    eng = nc.vector if (s + ob) % 2 == 0 else nc.gpsimd
                    v = sp[:].rearrange("p (r c) -> p r c", r=35, c=35)
                    eng.memset(sp[:, 0:70], 0.0)
                    eng.memset(sp[:, 1155:1225], 0.0)
                    eng.memset(v[:, 2:33, 0:2], 0.0)
                    eng.memset(v[:, 2:33, 33:35], 0.0)

            # ---- input DMAs, spread across 4 engine queues ----
            # conv_k critical path first (sync + vector), then wTs, spins.
            nc.vector.memset(warm[:], 0.0)
            for cb, deng in ((0, nc.sync), (1, nc.gpsimd)):
                for ob in range(2):
                    nc_src = wTk_d[cb][:, ob * 1152:(ob + 1) * 1152]
                    deng.dma_start(wTk[(cb, ob)][:], nc_src)
                deng.dma_start(xk[cb][:], xk_d[cb][:])
            for ob in range(2):
                nc.scalar.dma_start(bk[ob][:], bk_d[ob])
                nc.scalar.dma_start(bs[ob][:], bs_d[ob])
            nc.scalar.dma_start(m32[:], m32_d)
            for cb in range(2):
                nc.sync.dma_start(wTs[(cb, 0)][:], wTs_d[cb][:, 0:1152])
                nc.gpsimd.dma_start(wTs[(cb, 1)][:], wTs_d[cb][:, 1152:2304])

            spin_views = {}

            def prefetch_sample(s, deng):
                t_in = spin_pool.tile([128, 2 * 33 * 34], bf16,
                                      tag="spin", name=f"spin{s}")
                deng.dma_start(t_in[:], xs_d[s])
                for cb in range(2):
                    spin_views[(s, cb)] = t_in[
                        :, cb * 1122:(cb + 1) * 1122].rearrange(
                        "p (h w) -> p h w", h=33, w=34)

            spin_engs = [nc.sync, nc.scalar, nc.gpsimd]
            for s in range(SPC):
                prefetch_sample(s, spin_engs[s % 3])

            # ---- PE warmup: ~10 junk matmuls to flip HAM to 8/8 ----
            pwarm = psA.tile([128, 512], f32, tag="mm", name="pwarm")
            for w in range(10):
                PE(nc.tensor.matmul(pwarm[:, 0:512], warm[:, 0:128],
                                    warm[:, 128:640], start=True, stop=True))

            # ---- conv_k: all 8 samples batched on the free dim (N=200) ----
            for ob in range(2):
                pk = psA.tile([128, 512], f32, tag="mm", name=f"pk{ob}")
                idx = 0
                for cb in range(2):
                    for t in range(9):
                        PE(nc.tensor.matmul(
                            pk[:, 0:200],
                            wTk[(cb, ob)][:, t * 128:(t + 1) * 128],
                            xk[cb][:, t * 200:(t + 1) * 200],
                            start=(idx == 0), stop=(idx == 17)))
                        idx += 1
                nc.scalar.activation(kf[ob][:], pk[:, 0:200], RELU,
                                     bias=bk[ob][:, 0:1], scale=1.0)

            # ---- wx build (DVE): full-width diag-block xcorr weights ----
            # wx[X,ob][32i+c, t*128+32j+c'] = delta(c,c') * kf[ob][32i+c, (4X+j)*25+t]
            for X in range(2):
                for ob in range(2):
                    kfb = kf[ob][:, X * 100:(X + 1) * 100].rearrange(
                        "p (j t) -> p t j", j=4, t=25).unsqueeze(
                        -1).broadcast_to([128, 25, 4, 32])
                    nc.vector.tensor_tensor(
                        wx[(X, ob)][:], m32[:], kfb, mybir.AluOpType.mult)

            # ---- conv_s: half-quads (2 samples), 4 MMs per weight load,
            # alternating PSUM pools for stall-free eviction overlap ----
            conv_phase = 0
            for h in range(4):
                for ob in range(2):
                    pool = psA if conv_phase % 2 == 0 else psB
                    conv_phase += 1
                    ptiles = {}
                    for sl in range(2):
                        for ci in range(2):
                            ptiles[(sl, ci)] = pool.tile(
                                [128, 512], f32, tag="mm",
                                name=f"pc{h}_{ob}_{sl}_{ci}")
                    idx = 0
                    for cb in range(2):
                        for t in range(9):
                            dy, dx = t // 3, t % 3
                            lhsT = wTs[(cb, ob)][:, t * 128:(t + 1) * 128]
                            group = []
                            for sl in range(2):
                                s = h * 2 + sl
                                for ci, (y0, nr) in enumerate(CI_SPEC):
                                    mm = PE(nc.tensor.matmul(
                                        ptiles[(sl, ci)][:, 0:nr * 31],
                                        lhsT,
                                        spin_views[(s, cb)][:, y0 + dy:y0 + dy + nr,
                                                            dx:dx + 31],
                                        start=(idx == 0), stop=(idx == 17)))
                                    group.append(mm)
                            for mm in group[1:]:
                                mm.ldweights = False
                            noload_pairs.append((group[0], group[1:]))
                            idx += 1
                    for sl in range(2):
                        s = h * 2 + sl
                        sov = spout[(s, ob)][:].rearrange(
                            "p (h w) -> p h w", h=35, w=35)
                        for ci, (y0, nr) in enumerate(CI_SPEC):
                            pv = ptiles[(sl, ci)][:, 0:nr * 31].rearrange(
                                "p (h w) -> p h w", h=nr, w=31)
                            nc.scalar.activation(
                                sov[:, 2 + y0:2 + y0 + nr, 2:33],
                                pv[:, :, :], RELU,
                                bias=bs[ob][:, 0:1], scale=1.0)

            # ---- xcorr: 16x 32x32 diag tiles, one full LDW per tap ----
            def xcorr_phase(X, ob, ci, xo, pool):
                y0, nr = CI_SPEC[ci]
                N = nr * 31
                P = [pool.tile([128, 512], f32, tag="mm",
                               name=f"px{X}_{ob}_{ci}_{i}") for i in range(4)]
                sovs = [spout[(X * 4 + j, ob)][:].rearrange(
                    "p (h w) -> p h w", h=35, w=35) for j in range(4)]
                for t in range(25):
                    dy, dx = t // 5, t % 5
                    ldw = PE(nc.tensor.ldweights(
                        wx[(X, ob)][:, t * 128:(t + 1) * 128]))
                    mms = []
                    for i in range(4):
                        for j in range(4):
                            mm = PE(nc.tensor.matmul(
                                P[i][32 * j:32 * j + 32, 0:N],
                                wx[(X, ob)][32 * i:32 * i + 32,
                                            t * 128 + 32 * j:t * 128 + 32 * j + 32],
                                sovs[j][32 * i:32 * i + 32,
                                        y0 + dy:y0 + dy + nr, dx:dx + 31],
                                start=(t == 0), stop=(t == 24),
                                tile_position=(32 * i, 32 * j)))
                            mm.ldweights = False
                            mms.append(mm)
                    noload_pairs.append((ldw, list(mms)))
                ci_off = 0 if ci == 0 else 4 * 496
                for i in range(4):
                    if i % 2 == 0:
                        nc.scalar.copy(xo[:, ci_off + i * N:ci_off + (i + 1) * N],
                                       P[i][:, 0:N])
                    else:
                        nc.vector.tensor_copy(
                            xo[:, ci_off + i * N:ci_off + (i + 1) * N],
                            P[i][:, 0:N])

            xc_phase = 0
            for X in range(2):
                for ob in range(2):
                    xo = xout_pool.tile([128, 3844], bf16, tag="xo",
                                        name=f"xo{X}_{ob}")
                    for ci in range(2):
                        pool = psA if xc_phase % 2 == 0 else psB
                        xc_phase += 1
                        xcorr_phase(X, ob, ci, xo, pool)
                        ci_off = 0 if ci == 0 else 4 * 496
                        ln = 4 * CI_SPEC[ci][1] * 31
                        deng = nc.gpsimd if (X + ob) % 2 == 0 else nc.sync
                        deng.dma_start(out_d[X, ob, :, ci_off:ci_off + ln],
                                       xo[:, ci_off:ci_off + ln])

    noload_names = set()
    for _, mms in noload_pairs:
        for mm in mms:
            noload_names.add(mm.name)
    dropped = _drop_redundant_ldws(nc, noload_names)
    assert dropped == len(noload_names), (dropped, len(noload_names))
    nc.compile()
    _check_weight_safety(nc, noload_pairs)
    return nc


def _drop_redundant_ldws(nc, noload_names):
    """The tile scheduler splits every InstMatmult into LDWEIGHTS+MATMUL.
    For matmuls that reuse the already-loaded weights (marked
    ldweights=False at emission), drop the redundant split-out LDWEIGHTS.
    Those LDWs carry no sync_info and no deps, so removal is safe."""
    dropped = 0
    for f in nc.m.functions:
        for bb in f.blocks:
            insts = bb.instructions
            pe_pos = [k for k, i in enumerate(insts) if type(i).__name__
                      in ("InstLdweights", "InstMatmult")]
            drop = set()
            for a, b in zip(pe_pos, pe_pos[1:]):
                ia, ib = insts[a], insts[b]
                if (type(ia).__name__ == "InstLdweights"
                        and ia.sync_info is None
                        and not list(ia.nosync_dependency_names())
                        and type(ib).__name__ == "InstMatmult"
                        and ib.name in noload_names):
                    drop.add(a)
            if drop:
                bb.instructions = [i for k, i in enumerate(insts)
                                   if k not in drop]
                dropped += len(drop)
    return dropped


def _check_weight_safety(nc, noload_pairs):
    """Verify no weight-loading PE instruction lands between a loader and
    its ldweights=False dependents in the final scheduled order."""
    order = {}
    pos = 0
    for f in nc.m.functions:
        for bb in f.blocks:
            for ins in bb.instructions:
                if ins.engine == mybir.EngineType.PE:
                    order[ins.name] = (pos, ins)
                    pos += 1
    seq = sorted(order.values(), key=lambda t: t[0])
    loads_at = []
    for p, ins in seq:
        if type(ins).__name__ == "InstLdweights":
            loads_at.append((p, ins.name))
    import bisect
    for loader, mms in noload_pairs:
        if loader.name not in order:
            continue
        lp = order[loader.name][0]
        for mm in mms:
            mp = order[mm.name][0]
            assert mp > lp, f"noload MM {mm.name} scheduled before loader"
            idx = bisect.bisect_right([x[0] for x in loads_at], lp)
            while idx < len(loads_at) and loads_at[idx][0] < mp:
                bad = loads_at[idx]
                raise AssertionError(
                    f"weight clobber: {bad[1]} between {loader.name} and {mm.name}")


def _host_prep(kernel, search, w_k, g_k, b_k, m_k, v_k, w_s, g_s, b_s, m_s, v_s):
    import ml_dtypes
    bf16 = ml_dtypes.bfloat16

    def fold(w, g, b, m, v):
        scale = g / np.sqrt(v + EPS)
        return (w * scale[:, None, None, None]).astype(np.float32), \
               (b - m * scale).astype(np.float32)

    wkf, bias_k = fold(w_k, g_k, b_k, m_k, v_k)
    wsf, bias_s = fold(w_s, g_s, b_s, m_s, v_s)

    def packT(w):  # [o, ci, 3, 3] -> [cb, ci, (ob, t, o)] bf16
        arr = w.reshape(2, 128, 2, 128, 9).transpose(2, 3, 0, 4, 1)
        return np.ascontiguousarray(arr, dtype=np.float32).astype(
            bf16).reshape(2, 128, 2304)

    wTk = packT(wkf)
    wTs = packT(wsf)

    M32 = np.zeros((128, 32), dtype=np.float32)
    for p in range(128):
        M32[p, p % 32] = 1.0
    M32REP = np.tile(M32, (1, 100)).astype(bf16)  # [128, 3200] = 25t x 4j x 32

    bk = np.ascontiguousarray(bias_k.reshape(2, 128, 1))
    bs = np.ascontiguousarray(bias_s.reshape(2, 128, 1))

    in_maps = []
    for core in range(N_CORES):
        kin = kernel[core * SPC:(core + 1) * SPC]
        sin = search[core * SPC:(core + 1) * SPC]

        Xk = np.zeros((2, 128, 9, 200), dtype=np.float32)
        for t in range(9):
            dy, dx = t // 3, t % 3
            p = kin[:, :, dy:dy + 5, dx:dx + 5].reshape(SPC, 2, 128, 25)
            Xk[:, :, t, :] = p.transpose(1, 2, 0, 3).reshape(2, 128, 200)
        Xk = Xk.astype(bf16).reshape(2, 128, 1800)

        Xs = np.zeros((SPC, 2, 128, 33, 34), dtype=np.float32)
        Xs[:, :, :, 1:32, 1:32] = sin.reshape(SPC, 2, 128, 31, 31)
        Xs = np.ascontiguousarray(
            Xs.transpose(0, 2, 1, 3, 4)).astype(bf16).reshape(
            SPC, 128, 2 * 33 * 34)

        in_maps.append({
            "wTs0": wTs[0], "wTs1": wTs[1],
            "wTk0": wTk[0], "wTk1": wTk[1],
            "xk0": Xk[0], "xk1": Xk[1],
            "xs": Xs, "bk": bk, "bs": bs, "m32rep": M32REP,
        })
    return in_maps


def kernel(kernel, search, w_k, g_k, b_k, m_k, v_k, w_s, g_s, b_s, m_s, v_s,
           _trace=False):
    global _cached_nc, last_results
    args = [np.ascontiguousarray(np.asarray(x, dtype=np.float32)) for x in
            (kernel, search, w_k, g_k, b_k, m_k, v_k, w_s, g_s, b_s, m_s, v_s)]
    if _cached_nc is None:
        _cached_nc = _build_program()
    nc = _cached_nc
    in_maps = _host_prep(*args)
    res = run_bass_kernel_spmd(nc, in_maps, core_ids=list(range(N_CORES)),
                               trace=_trace)
    last_results = res
    outs = []
    for i in range(N_CORES):
        arr = np.asarray(res.results[i]["out"], dtype=np.float32)
        full = np.concatenate(
            [arr[..., :4 * 496].reshape(2, 2, 128, 4, 496),
             arr[..., 4 * 496:].reshape(2, 2, 128, 4, 465)],
            axis=-1)  # [X, ob, p=(j,c), i, 961]
        v = full.reshape(2, 2, 4, 32, 4, 961)  # X, ob, j, c, i, pos
        o = v.transpose(0, 2, 1, 4, 3, 5).reshape(SPC, 256, 31, 31)
        outs.append(o)
    out = np.concatenate(outs, axis=0)
    return np.ascontiguousarray(out)


# revision 20
# speedup vs baseline: 1.1212x; 1.0149x over previous
"""Trainium2 Bass kernel for DepthwiseXCorrAug.

Computes, for B=64 samples sharded 8-per-core across 8 NeuronCores:
  k = relu(bn(conv3x3_valid(kernel_in, w_k)))     # [B,256,5,5]
  s = relu(bn(conv3x3_same(search_in, w_s)))      # [B,256,31,31]
  out = per-sample per-channel xcorr(s, k), pad 2 # [B,256,31,31]

v2 design (per core), driven by NTFF trace analysis of v1 (245-292us):
  - PE instruction stream is made strictly in-order via an explicit
    same-engine dependency chain; this makes shared-weight matmul groups
    (InstMatmult.ldweights=False) safe against scheduler interleaving.
  - ~10 junk warmup matmuls at t=0 push the PE HAM clock gate to 8/8
    (2.4GHz) before real work arrives (saves the 1.2GHz cold phase).
  - conv branches: bf16 full-width matmuls, bias+ReLU by ScalarE on
    eviction. conv_s batches 4 samples x 2 ci per weight load; only the
    first matmul of each group self-loads weights (v1 paid a ~40ns/MM
    LDWEIGHTS tax on every matmul).
  - xcorr: 16 concurrent 32x32 diagonal tiles (4 samples x 4 channel
    quarters), ONE full-width 128-col LDWEIGHTS per tap loads all 16
    diag blocks (strips for 4 samples interleaved on the free dim);
    all 16 matmuls run with ldweights=False. 2x the per-tap throughput
    of v1's 2x2 64x64 scheme and ~1/32 the weight-load instructions.
  - kf (conv_k output) held bf16 so the DVE strip build has no casts.
  - outputs staged per (chunk, ob) into [128, 3844] tiles -> 8 fat DMAs
    with ~4KB per-partition descriptors (v1: 16 DMAs, 1.9KB rows, 24us
    unoverlapped tail); host does the final layout unshuffle.
"""

import sys

sys.path.insert(0, "/opt/trn_rl_repo")

import numpy as np

import concourse.bass as bass
import concourse.mybir as mybir
import concourse.tile as tile
from concourse import bacc
from concourse.bass_utils import run_bass_kernel_spmd
from concourse.tile_rust import add_dep_helper

EPS = 1e-5
N_CORES = 8
B, CIN, HID = 64, 256, 256
SPC = B // N_CORES  # samples per core
CI_SPEC = [(0, 16), (16, 15)]  # (y0, nr) output-row split, 961 = 496+465

_cached_nc = None
last_results = None  # set by kernel(); used by test harness for profiling


def _build_program():
    f32 = mybir.dt.float32
    bf16 = mybir.dt.bfloat16
    RELU = mybir.ActivationFunctionType.Relu

    nc = bacc.Bacc("TRN2", target_bir_lowering=False, debug=False,
                   num_devices=N_CORES)

    wTs_d = [nc.dram_tensor(f"wTs{cb}", [128, 2304], bf16, kind="ExternalInput").ap()
             for cb in range(2)]
    wTk_d = [nc.dram_tensor(f"wTk{cb}", [128, 2304], bf16, kind="ExternalInput").ap()
             for cb in range(2)]
    xk_d = [nc.dram_tensor(f"xk{cb}", [128, 1800], bf16, kind="ExternalInput").ap()
            for cb in range(2)]
    xs_d = nc.dram_tensor("xs", [SPC, 128, 2 * 33 * 34], bf16, kind="ExternalInput").ap()
    bk_d = nc.dram_tensor("bk", [2, 128, 1], f32, kind="ExternalInput").ap()
    bs_d = nc.dram_tensor("bs", [2, 128, 1], f32, kind="ExternalInput").ap()
    m32_d = nc.dram_tensor("m32rep", [128, 3200], bf16, kind="ExternalInput").ap()
    # out[X, ob, p=32j+c, (ci0: 4i x 496 | ci1: 4i x 465)]
    out_d = nc.dram_tensor("out", [2, 2, 128, 3844], bf16, kind="ExternalOutput").ap()

    pe_chain = []          # strict in-order PE instruction chain (mybir insts)
    noload_pairs = []      # (loader_inst, [mm_insts]) for post-compile check
    extra_noload_names = set()  # LDW-dropping only (warmups)

    def PE(binst):
        inst = binst.ins
        if pe_chain:
            add_dep_helper(inst, pe_chain[-1], reason="pe-inorder-chain")
        pe_chain.append(inst)
        return inst

    with tile.TileContext(nc) as tc:
        with tc.tile_pool(name="wp", bufs=1) as wp, \
             tc.tile_pool(name="spin", bufs=8) as spin_pool, \
             tc.tile_pool(name="spoutp", bufs=1) as spout_pool, \
             tc.tile_pool(name="xop", bufs=2) as xout_pool, \
             tc.tile_pool(name="psA", bufs=4, space="PSUM") as psA, \
             tc.tile_pool(name="psB", bufs=4, space="PSUM") as psB:

            # ---- persistent tiles ----
            wTs = {(cb, ob): wp.tile([128, 1152], bf16, tag=f"wTs{cb}{ob}",
                                     name=f"wTs{cb}{ob}")
                   for cb in range(2) for ob in range(2)}
            wTk = {(cb, ob): wp.tile([128, 1152], bf16, tag=f"wTk{cb}{ob}",
                                     name=f"wTk{cb}{ob}")
                   for cb in range(2) for ob in range(2)}
            xk = [wp.tile([128, 1800], bf16, tag=f"xk{cb}", name=f"xk{cb}")
                  for cb in range(2)]
            bk = [wp.tile([128, 1], f32, tag=f"bk{ob}", name=f"bk{ob}")
                  for ob in range(2)]
            bs = [wp.tile([128, 1], f32, tag=f"bs{ob}", name=f"bs{ob}")
                  for ob in range(2)]
            m32 = wp.tile([128, 3200], bf16, tag="m32", name="m32")
            kf = [wp.tile([128, 200], bf16, tag=f"kf{ob}", name=f"kf{ob}")
                  for ob in range(2)]
            wx = {(X, ob): wp.tile([128, 3200], bf16, tag=f"wx{X}{ob}",
                                   name=f"wx{X}{ob}")
                  for X in range(2) for ob in range(2)}
            warm = wp.tile([128, 640], bf16, tag="warm", name="warm")

            # ---- spout tiles: all 16 live (xcorr runs after all conv_s) ----
            spout = {}
            for s in range(SPC):
                for ob in range(2):
                    sp = spout_pool.tile([128, 35 * 35], bf16,
                                         tag=f"spout{s}_{ob}", name=f"spout{s}_{ob}")
                    spout[(s, ob)] = sp
                    eng = nc.vector if (s + ob) % 2 == 0 else nc.gpsimd
                    v = sp[:].rearrange("p (r c) -> p r c", r=35, c=35)
                    eng.memset(sp[:, 0:70], 0.0)
                    eng.memset(sp[:, 1155:1225], 0.0)
                    eng.memset(v[:, 2:33, 0:2], 0.0)
                    eng.memset(v[:, 2:33, 33:35], 0.0)

            # ---- input DMAs, spread across 4 engine queues ----
            # conv_k critical path first (sync + vector), then wTs, spins.
            nc.vector.memset(warm[:], 0.0)
            for cb, deng in ((0, nc.sync), (1, nc.gpsimd)):
                for ob in range(2):
                    nc_src = wTk_d[cb][:, ob * 1152:(ob + 1) * 1152]
                    deng.dma_start(wTk[(cb, ob)][:], nc_src)
                deng.dma_start(xk[cb][:], xk_d[cb][:])
            for ob in range(2):
                nc.scalar.dma_start(bk[ob][:], bk_d[ob])
                nc.scalar.dma_start(bs[ob][:], bs_d[ob])
            nc.scalar.dma_start(m32[:], m32_d)
            for cb in range(2):
                nc.sync.dma_start(wTs[(cb, 0)][:], wTs_d[cb][:, 0:1152])
                nc.gpsimd.dma_start(wTs[(cb, 1)][:], wTs_d[cb][:, 1152:2304])

            spin_views = {}

            def prefetch_sample(s, deng):
                t_in = spin_pool.tile([128, 2 * 33 * 34], bf16,
                                      tag="spin", name=f"spin{s}")
                deng.dma_start(t_in[:], xs_d[s])
                for cb in range(2):
                    spin_views[(s, cb)] = t_in[
                        :, cb * 1122:(cb + 1) * 1122].rearrange(
                        "p (h w) -> p h w", h=33, w=34)

            spin_engs = [nc.sync, nc.scalar, nc.gpsimd]
            for s in range(SPC):
                prefetch_sample(s, spin_engs[s % 3])

            # ---- PE warmup: junk matmuls keep the HAM clock gate at 8/8
            # during input-DMA stalls. Excluded from the PE chain and given
            # huge bass_priority so they only run when nothing real is ready;
            # all but the first skip weight loads (zero rhs -> harmless with
            # whatever weights are loaded).
            pwarm = psA.tile([128, 512], f32, tag="mm", name="pwarm")
            warm_mms = []
            for w in range(25):
                bm = nc.tensor.matmul(pwarm[:, 0:512], warm[:, 0:128],
                                      warm[:, 128:640], start=True, stop=True)
                warm_mms.append(bm.ins)
            for k, wm in enumerate(warm_mms):
                wm.bass_priority = 3_000_000 + k
                if k > 0:
                    wm.ldweights = False
                    extra_noload_names.add(wm.name)

            # ---- conv_k: all 8 samples batched on the free dim (N=200) ----
            for ob in range(2):
                pk = psA.tile([128, 512], f32, tag="mm", name=f"pk{ob}")
                idx = 0
                for cb in range(2):
                    for t in range(9):
                        PE(nc.tensor.matmul(
                            pk[:, 0:200],
                            wTk[(cb, ob)][:, t * 128:(t + 1) * 128],
                            xk[cb][:, t * 200:(t + 1) * 200],
                            start=(idx == 0), stop=(idx == 17)))
                        idx += 1
                nc.scalar.activation(kf[ob][:], pk[:, 0:200], RELU,
                                     bias=bk[ob][:, 0:1], scale=1.0)

            # ---- wx build (DVE): full-width diag-block xcorr weights ----
            # wx[X,ob][32i+c, t*128+32j+c'] = delta(c,c') * kf[ob][32i+c, (4X+j)*25+t]
            for X in range(2):
                for ob in range(2):
                    kfb = kf[ob][:, X * 100:(X + 1) * 100].rearrange(
                        "p (j t) -> p t j", j=4, t=25).unsqueeze(
                        -1).broadcast_to([128, 25, 4, 32])
                    nc.vector.tensor_tensor(
                        wx[(X, ob)][:], m32[:], kfb, mybir.AluOpType.mult)

            # ---- conv_s: half-quads (2 samples), 4 MMs per weight load,
            # alternating PSUM pools for stall-free eviction overlap ----
            conv_phase = 0
            for h in range(4):
                for ob in range(2):
                    pool = psA if conv_phase % 2 == 0 else psB
                    conv_phase += 1
                    ptiles = {}
                    for sl in range(2):
                        for ci in range(2):
                            ptiles[(sl, ci)] = pool.tile(
                                [128, 512], f32, tag="mm",
                                name=f"pc{h}_{ob}_{sl}_{ci}")
                    idx = 0
                    for cb in range(2):
                        for t in range(9):
                            dy, dx = t // 3, t % 3
                            lhsT = wTs[(cb, ob)][:, t * 128:(t + 1) * 128]
                            group = []
                            for sl in range(2):
                                s = h * 2 + sl
                                for ci, (y0, nr) in enumerate(CI_SPEC):
                                    mm = PE(nc.tensor.matmul(
                                        ptiles[(sl, ci)][:, 0:nr * 31],
                                        lhsT,
                                        spin_views[(s, cb)][:, y0 + dy:y0 + dy + nr,
                                                            dx:dx + 31],
                                        start=(idx == 0), stop=(idx == 17)))
                                    group.append(mm)
                            for mm in group[1:]:
                                mm.ldweights = False
                            noload_pairs.append((group[0], group[1:]))
                            idx += 1
                    for sl in range(2):
                        s = h * 2 + sl
                        sov = spout[(s, ob)][:].rearrange(
                            "p (h w) -> p h w", h=35, w=35)
                        for ci, (y0, nr) in enumerate(CI_SPEC):
                            pv = ptiles[(sl, ci)][:, 0:nr * 31].rearrange(
                                "p (h w) -> p h w", h=nr, w=31)
                            nc.scalar.activation(
                                sov[:, 2 + y0:2 + y0 + nr, 2:33],
                                pv[:, :, :], RELU,
                                bias=bs[ob][:, 0:1], scale=1.0)

            # ---- xcorr: 16x 32x32 diag tiles, one full-width LDWEIGHTS per
            # tap covering all 16 diag blocks; both ci halves per tap (8 PSUM
            # banks). The whole PE run per (X, ob) sits in a tile_critical
            # section: raw program order, no per-MM semaphore updates (which
            # serialized at ~26ns/inc and dominated the concurrent tile MMs).
            def xcorr_chunk(X, ob, xo):
                P = {}
                for ci in range(2):
                    pool = psA if ci == 0 else psB
                    for i in range(4):
                        P[(i, ci)] = pool.tile(
                            [128, 512], f32, tag="mm",
                            name=f"px{X}_{ob}_{ci}_{i}")
                sovs = [spout[(X * 4 + j, ob)][:].rearrange(
                    "p (h w) -> p h w", h=35, w=35) for j in range(4)]
                with tc.tile_critical(sync_engine=mybir.EngineType.PE,
                                      name=f"xc{X}{ob}"):
                    for t in range(25):
                        dy, dx = t // 5, t % 5
                        nc.tensor.ldweights(
                            wx[(X, ob)][:, t * 128:(t + 1) * 128])
                        for ci, (y0, nr) in enumerate(CI_SPEC):
                            for i in range(4):
                                for j in range(4):
                                    bm = nc.tensor.matmul(
                                        P[(i, ci)][32 * j:32 * j + 32, 0:nr * 31],
                                        wx[(X, ob)][32 * i:32 * i + 32,
                                                    t * 128 + 32 * j:
                                                    t * 128 + 32 * j + 32],
                                        sovs[j][32 * i:32 * i + 32,
                                                y0 + dy:y0 + dy + nr, dx:dx + 31],
                                        start=(t == 0), stop=(t == 24),
                                        tile_position=(32 * i, 32 * j))
                                    bm.ins.ldweights = False
                for ci, (y0, nr) in enumerate(CI_SPEC):
                    N = nr * 31
                    ci_off = 0 if ci == 0 else 4 * 496
                    for i in range(4):
                        dst = xo[:, ci_off + i * N:ci_off + (i + 1) * N]
                        if i % 2 == 0:
                            nc.scalar.copy(dst, P[(i, ci)][:, 0:N])
                        else:
                            nc.vector.tensor_copy(dst, P[(i, ci)][:, 0:N])

            for X in range(2):
                for ob in range(2):
                    xo = xout_pool.tile([128, 3844], bf16, tag="xo",
                                        name=f"xo{X}_{ob}")
                    xcorr_chunk(X, ob, xo)
                    for ci in range(2):
                        ci_off = 0 if ci == 0 else 4 * 496
                        ln = 4 * CI_SPEC[ci][1] * 31
                        deng = nc.gpsimd if (X + ob) % 2 == 0 else nc.sync
                        deng.dma_start(out_d[X, ob, :, ci_off:ci_off + ln],
                                       xo[:, ci_off:ci_off + ln])

    noload_names = set(extra_noload_names)
    for _, mms in noload_pairs:
        for mm in mms:
            noload_names.add(mm.name)
    dropped = _drop_redundant_ldws(nc, noload_names)
    assert dropped == len(noload_names), (dropped, len(noload_names))
    nc.compile()
    _check_weight_safety(nc, noload_pairs)
    return nc


def _drop_redundant_ldws(nc, noload_names):
    """The tile scheduler splits every InstMatmult into LDWEIGHTS+MATMUL.
    For matmuls that reuse the already-loaded weights (marked
    ldweights=False at emission), drop the redundant split-out LDWEIGHTS.
    Those LDWs carry no sync_info and no deps, so removal is safe."""
    dropped = 0
    for f in nc.m.functions:
        for bb in f.blocks:
            insts = bb.instructions
            pe_pos = [k for k, i in enumerate(insts) if type(i).__name__
                      in ("InstLdweights", "InstMatmult")]
            drop = set()
            for a, b in zip(pe_pos, pe_pos[1:]):
                ia, ib = insts[a], insts[b]
                if (type(ia).__name__ == "InstLdweights"
                        and ia.sync_info is None
                        and not list(ia.nosync_dependency_names())
                        and type(ib).__name__ == "InstMatmult"
                        and ib.name in noload_names):
                    drop.add(a)
            if drop:
                bb.instructions = [i for k, i in enumerate(insts)
                                   if k not in drop]
                dropped += len(drop)
    return dropped


def _check_weight_safety(nc, noload_pairs):
    """Verify no weight-loading PE instruction lands between a loader and
    its ldweights=False dependents in the final scheduled order."""
    order = {}
    pos = 0
    for f in nc.m.functions:
        for bb in f.blocks:
            for ins in bb.instructions:
                if ins.engine == mybir.EngineType.PE:
                    order[ins.name] = (pos, ins)
                    pos += 1
    seq = sorted(order.values(), key=lambda t: t[0])
    loads_at = []
    for p, ins in seq:
        if type(ins).__name__ == "InstLdweights":
            loads_at.append((p, ins.name))
    import bisect
    for loader, mms in noload_pairs:
        if loader.name not in order:
            continue
        lp = order[loader.name][0]
        for mm in mms:
            mp = order[mm.name][0]
            assert mp > lp, f"noload MM {mm.name} scheduled before loader"
            idx = bisect.bisect_right([x[0] for x in loads_at], lp)
            while idx < len(loads_at) and loads_at[idx][0] < mp:
                bad = loads_at[idx]
                raise AssertionError(
                    f"weight clobber: {bad[1]} between {loader.name} and {mm.name}")


def _host_prep(kernel, search, w_k, g_k, b_k, m_k, v_k, w_s, g_s, b_s, m_s, v_s):
    import ml_dtypes
    bf16 = ml_dtypes.bfloat16

    def fold(w, g, b, m, v):
        scale = g / np.sqrt(v + EPS)
        return (w * scale[:, None, None, None]).astype(np.float32), \
               (b - m * scale).astype(np.float32)

    wkf, bias_k = fold(w_k, g_k, b_k, m_k, v_k)
    wsf, bias_s = fold(w_s, g_s, b_s, m_s, v_s)

    def packT(w):  # [o, ci, 3, 3] -> [cb, ci, (ob, t, o)] bf16
        arr = w.reshape(2, 128, 2, 128, 9).transpose(2, 3, 0, 4, 1)
        return np.ascontiguousarray(arr, dtype=np.float32).astype(
            bf16).reshape(2, 128, 2304)

    wTk = packT(wkf)
    wTs = packT(wsf)

    M32 = np.zeros((128, 32), dtype=np.float32)
    for p in range(128):
        M32[p, p % 32] = 1.0
    M32REP = np.tile(M32, (1, 100)).astype(bf16)  # [128, 3200] = 25t x 4j x 32

    bk = np.ascontiguousarray(bias_k.reshape(2, 128, 1))
    bs = np.ascontiguousarray(bias_s.reshape(2, 128, 1))

    in_maps = []
    for core in range(N_CORES):
        kin = kernel[core * SPC:(core + 1) * SPC]
        sin = search[core * SPC:(core + 1) * SPC]

        Xk = np.zeros((2, 128, 9, 200), dtype=np.float32)
        for t in range(9):
            dy, dx = t // 3, t % 3
            p = kin[:, :, dy:dy + 5, dx:dx + 5].reshape(SPC, 2, 128, 25)
            Xk[:, :, t, :] = p.transpose(1, 2, 0, 3).reshape(2, 128, 200)
        Xk = Xk.astype(bf16).reshape(2, 128, 1800)

        Xs = np.zeros((SPC, 2, 128, 33, 34), dtype=np.float32)
        Xs[:, :, :, 1:32, 1:32] = sin.reshape(SPC, 2, 128, 31, 31)
        Xs = np.ascontiguousarray(
            Xs.transpose(0, 2, 1, 3, 4)).astype(bf16).reshape(
            SPC, 128, 2 * 33 * 34)

        in_maps.append({
            "wTs0": wTs[0], "wTs1": wTs[1],
            "wTk0": wTk[0], "wTk1": wTk[1],
            "xk0": Xk[0], "xk1": Xk[1],
            "xs": Xs, "bk": bk, "bs": bs, "m32rep": M32REP,
        })
    return in_maps


def kernel(kernel, search, w_k, g_k, b_k, m_k, v_k, w_s, g_s, b_s, m_s, v_s,
           _trace=False):
    global _cached_nc, last_results
    args = [np.ascontiguousarray(np.asarray(x, dtype=np.float32)) for x in
            (kernel, search, w_k, g_k, b_k, m_k, v_k, w_s, g_s, b_s, m_s, v_s)]
    if _cached_nc is None:
        _cached_nc = _build_program()
    nc = _cached_nc
    in_maps = _host_prep(*args)
    res = run_bass_kernel_spmd(nc, in_maps, core_ids=list(range(N_CORES)),
                               trace=_trace)
    last_results = res
    outs = []
    for i in range(N_CORES):
        arr = np.asarray(res.results[i]["out"], dtype=np.float32)
        full = np.concatenate(
            [arr[..., :4 * 496].reshape(2, 2, 128, 4, 496),
             arr[..., 4 * 496:].reshape(2, 2, 128, 4, 465)],
            axis=-1)  # [X, ob, p=(j,c), i, 961]
        v = full.reshape(2, 2, 4, 32, 4, 961)  # X, ob, j, c, i, pos
        o = v.transpose(0, 2, 1, 4, 3, 5).reshape(SPC, 256, 31, 31)
        outs.append(o)
    out = np.concatenate(outs, axis=0)
    return np.ascontiguousarray(out)
